# revision 44
# baseline (speedup 1.0000x reference)
"""2-layer relational GCN (RGCN) on Trainium2, 8-core SPMD.

Sharding: edges are partitioned by dst-node range (core c owns dst nodes
[c*N/8, (c+1)*N/8)); node features and per-relation weights are replicated.
Self-loops are folded in as an extra relation.

Per core:
  Phase 1 (layer 1, fused): dst tiles are split into two halves so the
    per-tile accumulators fit PSUM. Edges are grouped by (half, etype) with
    each group padded to 128-row chunks and sorted by dst tile within the
    group. Per chunk: indirect-DMA gather of feat[src] rows -> PE transpose ->
    matmul with W1[r] -> one-hot selection matmuls accumulate messages
    directly into the PSUM accumulator slice of each touched dst tile
    (no DRAM message buffer, no indirect scatter). Epilogue per half:
    +bias, ReLU -> h shard.
  AllGather h shards -> full h on every core.
  Phase 3 (layer 2, fused): gather h[src] rows in dst order, transform by all
    relations at once (W2 flattened), mask-select by etype, one-hot aggregate,
    +bias -> output shard. Host concatenates the 8 shards.

Indirect DMAs are limited to 128 descriptors each: the SWDGE ucode truncates
larger batches unless the descriptors coalesce (verified on HW: random-index
batches of k*128 descriptors land only 128 correctly).
"""

import numpy as np

P = 128          # partitions / tile edge
C = 8            # NeuronCores
SELB = 8         # one-hot selection matrices built per DVE op
DEBUG = False    # add h dump outputs
NO_COLLECTIVE = False  # replace AllGather with a local copy (TimelineSim)
PHASE_LIMIT = 3        # TimelineSim attribution: 1=layer1, 2=+collective,
                       # 3=full

_CACHE = {}


# ---------------------------------------------------------------- host prep

def _preprocess(feat, W1, loop1, b1, W2, loop2, b2, src, dst, etype):
    feat = np.ascontiguousarray(np.asarray(feat, dtype=np.float32))
    W1 = np.asarray(W1, dtype=np.float32)
    W2 = np.asarray(W2, dtype=np.float32)
    loop1 = np.asarray(loop1, dtype=np.float32)
    loop2 = np.asarray(loop2, dtype=np.float32)
    b1 = np.asarray(b1, dtype=np.float32)
    b2 = np.asarray(b2, dtype=np.float32)
    src = np.asarray(src).astype(np.int64).ravel()
    dst = np.asarray(dst).astype(np.int64).ravel()
    etype = np.asarray(etype).astype(np.int64).ravel()

    N, D = feat.shape
    R, _, H = W1.shape
    O = W2.shape[2]
    assert D == P and N % C == 0, (N, D)
    S = N // C                       # dst nodes per core
    NT = -(-S // P)                  # node tiles per core
    NT0 = (NT + 1) // 2              # tiles in half 0
    Rp = R + 1                       # +1 self-loop relation

    # append self-loop edges (relation R)
    sl = np.arange(N, dtype=np.int64)
    asrc = np.concatenate([src, sl])
    adst = np.concatenate([dst, sl])
    aet = np.concatenate([etype, np.full(N, R, dtype=np.int64)])

    core_of = adst // S
    HK = 32768                                   # int16 dma_gather limit

    per_core = []
    cellcnt = np.zeros((C, Rp, NT, 2), np.int64)  # (core, rel, tile, range)
    cnt2 = np.zeros((C, NT), np.int64)
    for c in range(C):
        m = core_of == c
        es, ed, ee = asrc[m], adst[m], aet[m]
        tl = ed - c * S
        tid = tl // P
        np.add.at(cellcnt[c], (ee, tid, (es >= HK).astype(np.int64)), 1)
        cnt2[c] = np.bincount(tid, minlength=NT)
        per_core.append((es, ee, tl, tid))

    # shared (core-independent) phase-1 layout: per-(rel, tile, src-range)
    # cell capacities = max over cores; cells laid out (half, rel, range,
    # tile); each (half, rel, range) block padded to a multiple of 128 so
    # dma_gather windows are single-range.
    cellcap = cellcnt.max(axis=0)                 # [Rp, NT, 2]
    halves = ([t for t in range(NT0)], [t for t in range(NT0, NT)])
    cell_off = np.zeros((Rp, NT, 2), np.int64)
    pos = 0
    tile_rel = []
    tile_half = []
    rpairs = []                                   # real pairs [chunk, tile]
    pair_bounds = []                              # (slot_lo, slot_hi)
    half_chunk0 = [None, None]
    windows1 = [[], []]                           # per half: (w0, nw, ishi)
    WMAX = 8
    for hf in (0, 1):
        half_chunk0[hf] = pos // P
        for r in range(Rp):
            for rng in (0, 1):
                bstart = pos
                for t in halves[hf]:
                    cap = int(cellcap[r, t, rng])
                    cell_off[r, t, rng] = pos
                    if cap == 0:
                        continue
                    a = pos
                    while a < pos + cap:
                        b = min(pos + cap, (a // P + 1) * P)
                        rpairs.append([a // P, t])
                        pair_bounds.append((a, b))
                        a = b
                    pos += cap
                pos = bstart + (-(-(pos - bstart) // P)) * P  # pad block
                nchunks = (pos - bstart) // P
                tile_rel += [r] * nchunks
                tile_half += [hf] * nchunks
                b0 = bstart // P
                for a in range(0, nchunks, WMAX):
                    windows1[hf].append((b0 + a, min(WMAX, nchunks - a),
                                         rng))
    T1 = pos // P
    # PSUM start=True clears has_written for the WHOLE bank, so per-tile
    # slices sharing a bank would corrupt each other. Instead: one synthetic
    # all-zero start=True pair per tile at the head of each half (clears the
    # banks before any real accumulation), then every real pair accumulates
    # with start=False (first real write overwrites since its bit is clear).
    last_of = {}
    for q, (ch, t) in enumerate(rpairs):
        last_of[t] = q
    assert len(last_of) == NT
    pairs = []                       # (chunk, tile, start, stop)
    real_final_idx = np.empty(len(rpairs), np.int64)
    qf = 0
    for hf in (0, 1):
        for t in halves[hf]:
            pairs.append((half_chunk0[hf], t, True, False))
            qf += 1
        for q, (ch, t) in enumerate(rpairs):
            if tile_half[ch] != hf:
                continue
            pairs.append((ch, t, False, q == last_of[t]))
            real_final_idx[q] = qf
            qf += 1
    NPAIR = len(pairs)
    pair_lo = np.array([a for a, _ in pair_bounds], np.int64)

    # layer-2 slots: per tile [lo-src segment | hi-src segment], each padded
    # to a 128-multiple of the cross-core max, so every dma_gather window is
    # single-range with all-valid int16 indices (src or src-HK).
    HK = 32768
    cnt_lo = np.zeros((C, NT), np.int64)
    cnt_hi = np.zeros((C, NT), np.int64)
    for c in range(C):
        es, ee, tl, tid = per_core[c]
        lo = es < HK
        np.add.at(cnt_lo[c], tid[lo], 1)
        np.add.at(cnt_hi[c], tid[~lo], 1)
    caplo = (-(-cnt_lo.max(0) // P)) * P
    caphi = (-(-cnt_hi.max(0) // P)) * P
    k_t = (caplo + caphi) // P                    # layer-2 chunks per tile
    off2 = np.concatenate([[0], np.cumsum(k_t * P)])
    L2 = int(off2[-1])                            # layer-2 slot count
    T2 = L2 // P

    chunk_tile = []
    chunk_k = []
    for t in range(NT):
        for k in range(int(k_t[t])):
            chunk_tile.append(t)
            chunk_k.append(k)
    windows = []                                  # (chunk0, nchunks, is_hi)
    WMAX = 8                                      # 1024-idx dma_gather limit
    for t in range(NT):
        base = int(off2[t]) // P
        nlo = int(caplo[t]) // P
        nhi = int(caphi[t]) // P
        for a in range(0, nlo, WMAX):
            windows.append((base + a, min(WMAX, nlo - a), 0))
        for a in range(0, nhi, WMAX):
            windows.append((base + nlo + a, min(WMAX, nhi - a), 1))

    # replicated tensors
    W1e = np.concatenate([W1, loop1[None]], axis=0)          # [Rp, D, H]
    W2e = np.concatenate([W2, loop2[None]], axis=0)          # [Rp, H, O]
    w1f = np.ascontiguousarray(W1e.transpose(1, 0, 2).reshape(D, Rp * H))
    w2f = np.ascontiguousarray(W2e.transpose(1, 0, 2).reshape(H, Rp * O))
    b1b = np.ascontiguousarray(np.broadcast_to(b1, (P, H)))
    b2b = np.ascontiguousarray(np.broadcast_to(b2, (P, O)))

    in_maps = []
    for c in range(C):
        es, ee, tl, tid = per_core[c]
        nE = len(es)
        # phase-1 slot: cell_off[(ee, tid, rng)] + rank within cell
        rng1 = (es >= HK).astype(np.int64)
        o1 = np.lexsort((es, tid, rng1, ee))
        key = ((ee * 2 + rng1) * NT + tid)[o1]
        starts = np.searchsorted(key, np.arange(Rp * 2 * NT), side="left")
        rank = np.arange(nE) - starts[key]
        slot1_sorted = cell_off[ee[o1], tid[o1], rng1[o1]] + rank
        # map each edge to its pair (pairs tile the cell slot space)
        pair_of = real_final_idx[
            np.searchsorted(pair_lo, slot1_sorted, side="right") - 1]
        g1val = np.zeros(T1 * P, np.int16)        # pads -> row 0 (valid)
        g1val[slot1_sorted] = (es - HK * rng1)[o1].astype(np.int16)
        s1i = np.arange(T1 * P)
        gi1 = np.zeros((16, T1 * 8), np.int16)
        gi1[s1i % 16, (s1i // 128) * 8 + (s1i % 128) // 16] = g1val
        g1w = np.ascontiguousarray(np.tile(gi1, (8, 1)))
        d1 = np.full((NPAIR, P), -1.0, np.float32)
        d1[pair_of, slot1_sorted % P] = (tl[o1] % P).astype(np.float32)

        # layer-2 slots: per (tile, src-range) segment, sorted by src
        lo2 = es < HK
        o2 = np.lexsort((es, ~lo2, tid))
        key2 = (tid * 2 + (~lo2).astype(np.int64))[o2]
        starts2 = np.searchsorted(key2, np.arange(2 * NT), side="left")
        rank2 = np.arange(nE) - starts2[key2]
        seg_off = np.empty(2 * NT, np.int64)
        seg_off[0::2] = off2[:-1]
        seg_off[1::2] = off2[:-1] + caplo
        slot2_sorted = seg_off[key2] + rank2
        slot2 = np.empty(nE, np.int64)
        slot2[o2] = slot2_sorted

        d2 = np.full(L2, -1.0, np.float32)
        e3 = np.full(L2, -1.0, np.float32)
        d2[slot2] = (tl % P).astype(np.float32)
        e3[slot2] = ee.astype(np.float32)
        gval = np.zeros(L2, np.int16)             # pads -> row 0 (valid)
        gval[slot2] = (es - HK * (~lo2)).astype(np.int16)
        # dma_gather index layout: slot s -> [s%16, (s//128)*8 + (s%128)//16]
        sidx = np.arange(L2)
        gi = np.zeros((16, T2 * 8), np.int16)
        gi[sidx % 16, (sidx // 128) * 8 + (sidx % 128) // 16] = gval
        g3i = np.ascontiguousarray(np.tile(gi, (8, 1)))

        def tr(a, T):
            return np.ascontiguousarray(a.reshape(T, P).T)

        in_maps.append({
            "feat": feat, "w1f": w1f, "w2f": w2f, "b1b": b1b, "b2b": b2b,
            "g1w": g1w,
            "d1t": np.ascontiguousarray(d1.T),    # [P, NPAIR]
            "d2t": tr(d2, T2), "g3i": g3i, "e3t": tr(e3, T2),
        })

    plan = dict(N=N, D=D, H=H, O=O, Rp=Rp, S=S, NT=NT, NT0=NT0, T1=T1, T2=T2,
                NPAIR=NPAIR, HK=HK,
                tile_rel=tuple(tile_rel), tile_half=tuple(tile_half),
                chunk_tile=tuple(chunk_tile), chunk_k=tuple(chunk_k),
                k_t=tuple(int(x) for x in k_t),
                windows=tuple(windows),
                windows1=(tuple(windows1[0]), tuple(windows1[1])),
                pairs=tuple(pairs))
    return plan, in_maps


# ---------------------------------------------------------------- device prog

def _bc_inner(ap, n):
    """[P, c] -> [P, c, n], broadcasting the new innermost dim."""
    import concourse.bass as bass
    return bass.AP(ap.tensor, ap.offset, list(ap.ap) + [[0, n]])


def _bc_mid(ap, g):
    """[P, f] -> [P, g, f], broadcasting the new middle dim."""
    import concourse.bass as bass
    a = list(ap.ap)
    return bass.AP(ap.tensor, ap.offset, [a[0], [0, g]] + a[1:])


def _build(plan):
    import concourse.bacc as bacc
    import concourse.tile as tile
    import concourse.mybir as mybir
    from concourse.bass import IndirectOffsetOnAxis
    from concourse.masks import make_identity

    N, D, H, O, Rp = plan["N"], plan["D"], plan["H"], plan["O"], plan["Rp"]
    S, NT, NT0, T1, T2 = (plan["S"], plan["NT"], plan["NT0"], plan["T1"],
                          plan["T2"])
    NPAIR, HK = plan["NPAIR"], plan["HK"]
    tile_rel, tile_half = plan["tile_rel"], plan["tile_half"]
    chunk_tile, chunk_k, k_t = (plan["chunk_tile"], plan["chunk_k"],
                                plan["k_t"])
    pairs = plan["pairs"]
    windows = plan["windows"]
    windows1 = plan["windows1"]
    f32 = mybir.dt.float32
    bf16 = mybir.dt.bfloat16
    i32 = mybir.dt.int32
    i16 = mybir.dt.int16
    AO = mybir.AluOpType

    nc = bacc.Bacc("TRN2", target_bir_lowering=False, debug=False,
                   num_devices=C)
    feat = nc.dram_tensor("feat", [N, D], f32, kind="ExternalInput")
    w1f = nc.dram_tensor("w1f", [D, Rp * H], f32, kind="ExternalInput")
    w2f = nc.dram_tensor("w2f", [H, Rp * O], f32, kind="ExternalInput")
    b1b = nc.dram_tensor("b1b", [P, H], f32, kind="ExternalInput")
    b2b = nc.dram_tensor("b2b", [P, O], f32, kind="ExternalInput")
    g1wt = nc.dram_tensor("g1w", [P, T1 * 8], i16, kind="ExternalInput")
    d1t = nc.dram_tensor("d1t", [P, NPAIR], f32, kind="ExternalInput")
    d2t = nc.dram_tensor("d2t", [P, T2], f32, kind="ExternalInput")
    g3i = nc.dram_tensor("g3i", [P, T2 * 8], i16, kind="ExternalInput")
    e3t = nc.dram_tensor("e3t", [P, T2], f32, kind="ExternalInput")
    outs = nc.dram_tensor("out_shard", [S, O], f32, kind="ExternalOutput")
    dbg_h = dbg_hf = None
    if DEBUG:
        dbg_h = nc.dram_tensor("dbg_h", [S, H], f32, kind="ExternalOutput")
        dbg_hf = nc.dram_tensor("dbg_hf", [N, H], f32, kind="ExternalOutput")

    with tile.TileContext(nc) as tc:
        with tc.tile_pool(name="dram", bufs=1, space="DRAM") as dramp:
            h_shard = dramp.tile([NT * P, H], f32, name="h_shard")
            h_full = dramp.tile([N, H], f32, addr_space="Shared", name="h_full")

            with tc.tile_pool(name="const", bufs=1) as cp:
                ident = cp.tile([P, P], f32, name="ident")
                make_identity(nc, ident[:])
                iota_i = cp.tile([P, P], i32, name="iota_i")
                nc.gpsimd.iota(iota_i[:], pattern=[[1, P]], base=0,
                               channel_multiplier=0)
                iota_f = cp.tile([P, P], f32, name="iota_f")
                nc.vector.tensor_copy(iota_f[:], iota_i[:])
                c40_i = cp.tile([P, Rp * O], i32, name="c40_i")
                nc.gpsimd.iota(c40_i[:], pattern=[[1, Rp], [0, O]], base=0,
                               channel_multiplier=0)
                c40_f = cp.tile([P, Rp * O], f32, name="c40_f")
                nc.vector.tensor_copy(c40_f[:], c40_i[:])
                w1s = cp.tile([D, Rp * H], f32, name="w1s")
                nc.sync.dma_start(out=w1s[:], in_=w1f[:])
                w2s = cp.tile([H, Rp * O], f32, name="w2s")
                nc.sync.dma_start(out=w2s[:], in_=w2f[:])
                b1s = cp.tile([P, H], f32, name="b1s")
                nc.sync.dma_start(out=b1s[:], in_=b1b[:])
                b2s = cp.tile([P, O], f32, name="b2s")
                nc.sync.dma_start(out=b2s[:], in_=b2b[:])
                g1s = cp.tile([P, T1 * 8], i16, name="g1s")
                nc.sync.dma_start(out=g1s[:], in_=g1wt[:])
                d1s = cp.tile([P, NPAIR], f32, name="d1s")
                nc.sync.dma_start(out=d1s[:], in_=d1t[:])
                d2s = cp.tile([P, T2], f32, name="d2s")
                nc.sync.dma_start(out=d2s[:], in_=d2t[:])
                g3s = cp.tile([P, T2 * 8], i16, name="g3s")
                nc.sync.dma_start(out=g3s[:], in_=g3i[:])
                e3s = cp.tile([P, T2], f32, name="e3s")
                nc.sync.dma_start(out=e3s[:], in_=e3t[:])

                # ---------------- phase 1: layer 1 fused --------------------
                from concourse import library_config
                nc.gpsimd.load_library(library_config.mlp)
                # pair index ranges per chunk
                pair_by_chunk = {}
                for q, pr in enumerate(pairs):
                    pair_by_chunk.setdefault(pr[0], []).append(q)

                with tc.tile_pool(name="p1sb", bufs=12) as sb, \
                     tc.tile_pool(name="p1hg", bufs=3) as hgp:
                    stage_of_chunk = {}
                    for hf in range(2):
                        tbase = 0 if hf == 0 else NT0
                        nth = NT0 if hf == 0 else NT - NT0
                        if PHASE_LIMIT < 1:
                            break
                        with tc.tile_pool(name=f"accp{hf}", bufs=1,
                                          space="PSUM") as accp, \
                             tc.tile_pool(name=f"p1ps{hf}", bufs=2,
                                          space="PSUM") as psp:
                            acc = accp.tile([P, nth * H], f32, name="acc")
                            pend_pairs = []

                            def flush_pairs():
                                if not pend_pairs:
                                    return
                                nbq = len(pend_pairs)
                                q0 = pend_pairs[0]
                                selb = sb.tile([P, nbq * P], bf16, tag="selb",
                                               name="selb")
                                nc.vector.tensor_tensor(
                                    out=selb[:].rearrange(
                                        "p (g j) -> p g j", g=nbq),
                                    in0=_bc_inner(d1s[:, q0:q0 + nbq], P),
                                    in1=_bc_mid(iota_f[:], nbq),
                                    op=AO.is_equal)
                                for i, q in enumerate(pend_pairs):
                                    ch, t, st, sp = pairs[q]
                                    nc.tensor.matmul(
                                        out=acc[:, (t - tbase) * H:
                                                (t - tbase + 1) * H],
                                        lhsT=selb[:, i * P:(i + 1) * P],
                                        rhs=stage_of_chunk[ch][:],
                                        start=st, stop=sp)
                                pend_pairs.clear()

                            for w0, nw, ishi in windows1[hf]:
                                hgw = hgp.tile([P, nw * D], f32, tag="hgw",
                                               name="hgw")
                                srcw = feat[HK:N, :] if ishi else feat[0:HK, :]
                                nc.gpsimd.dma_gather(
                                    hgw[:].rearrange("p (c e) -> p c e",
                                                     c=nw),
                                    srcw, g1s[:, w0 * 8:(w0 + nw) * 8],
                                    nw * P, nw * P, D)
                                for j in range(nw):
                                    t1 = w0 + j
                                    r = tile_rel[t1]
                                    gtp = psp.tile([P, P], f32, tag="gtp",
                                                   name="gtp")
                                    msp = psp.tile([P, H], f32, tag="msp",
                                                   name="msp")
                                    nc.tensor.transpose(
                                        out=gtp[:],
                                        in_=hgw[:, j * D:(j + 1) * D],
                                        identity=ident[:])
                                    gts = sb.tile([P, P], f32, tag="gts",
                                                  name="gts")
                                    nc.scalar.copy(out=gts[:], in_=gtp[:])
                                    nc.tensor.matmul(
                                        out=msp[:], lhsT=gts[:],
                                        rhs=w1s[:, r * H:(r + 1) * H],
                                        start=True, stop=True)
                                    stage = sb.tile([P, H], bf16, tag="stage",
                                                    name="stage")
                                    nc.scalar.copy(out=stage[:], in_=msp[:])
                                    stage_of_chunk[t1] = stage
                                    for q in pair_by_chunk.get(t1, []):
                                        pend_pairs.append(q)
                                        if len(pend_pairs) == SELB:
                                            flush_pairs()
                            flush_pairs()
                            # epilogue: bias + relu -> h_shard
                            hb = sb.tile([P, nth * H], f32, tag="hb",
                                         name="hb")
                            nc.vector.tensor_tensor(
                                out=hb[:].rearrange("p (t h) -> p t h",
                                                    t=nth),
                                in0=acc[:].rearrange("p (t h) -> p t h",
                                                     t=nth),
                                in1=_bc_mid(b1s[:], nth),
                                op=AO.add)
                            nc.vector.tensor_scalar_max(
                                out=hb[:], in0=hb[:], scalar1=0.0)
                            nc.sync.dma_start(
                                out=h_shard[tbase * P:(tbase + nth) * P, :]
                                .rearrange("(t p) h -> p t h", p=P),
                                in_=hb[:].rearrange("p (t h) -> p t h",
                                                    t=nth))

                    if PHASE_LIMIT < 2:
                        pass
                    elif NO_COLLECTIVE:
                        nc.sync.dma_start(out=h_full[0:S, :],
                                          in_=h_shard[0:S, :])
                    else:
                        nc.gpsimd.collective_compute(
                            "AllGather", AO.bypass,
                            replica_groups=[list(range(C))],
                            ins=[h_shard[0:S, :].opt()],
                            outs=[h_full[:].opt()])
                    if DEBUG:
                        nc.sync.dma_start(out=dbg_h[:], in_=h_shard[0:S, :])
                        nc.sync.dma_start(out=dbg_hf[:], in_=h_full[:])

                # ---------------- phase 3: layer 2 (fused) ------------------
                from concourse import library_config
                with tc.tile_pool(name="p3sb", bufs=8) as sb3, \
                     tc.tile_pool(name="p3ps", bufs=2, space="PSUM") as ps3:
                    if PHASE_LIMIT >= 3:
                        nc.gpsimd.load_library(library_config.mlp)
                    cur_otp = None
                    for u0, nb, ishi in (windows if PHASE_LIMIT >= 3 else []):
                        hg = sb3.tile([P, nb * H], f32, tag="hg", name="hg")
                        src = h_full[HK:N, :] if ishi else h_full[0:HK, :]
                        nc.gpsimd.dma_gather(
                            hg[:].rearrange("p (c e) -> p c e", c=nb),
                            src, g3s[:, u0 * 8:(u0 + nb) * 8],
                            nb * P, nb * P, H)
                        hgtp = ps3.tile([H, nb * P], f32, tag="hgtp",
                                        name="hgtp")
                        for j in range(nb):
                            nc.tensor.transpose(
                                out=hgtp[:, j * P:(j + 1) * P],
                                in_=hg[:, j * H:(j + 1) * H],
                                identity=ident[:])
                        hgt = sb3.tile([H, nb * P], f32, tag="hgt", name="hgt")
                        nc.scalar.copy(out=hgt[:], in_=hgtp[:])
                        m40 = ps3.tile([P, nb * Rp * O], f32, tag="m40",
                                       name="m40")
                        for j in range(nb):
                            nc.tensor.matmul(
                                out=m40[:, j * Rp * O:(j + 1) * Rp * O],
                                lhsT=hgt[:, j * P:(j + 1) * P], rhs=w2s[:],
                                start=True, stop=True)
                        mskb = sb3.tile([P, nb * Rp * O], f32, tag="mskb",
                                        name="mskb")
                        nc.vector.tensor_tensor(
                            out=mskb[:].rearrange("p (g c) -> p g c", g=nb),
                            in0=_bc_inner(e3s[:, u0:u0 + nb], Rp * O),
                            in1=_bc_mid(c40_f[:], nb),
                            op=AO.is_equal)
                        nc.vector.tensor_tensor(
                            out=mskb[:], in0=mskb[:], in1=m40[:], op=AO.mult)
                        m2b = sb3.tile([P, nb * O], f32, tag="m2b", name="m2b")
                        nc.vector.tensor_reduce(
                            out=m2b[:],
                            in_=mskb[:].rearrange("p (g r o) -> p g o r",
                                                  g=nb, r=Rp, o=O),
                            axis=mybir.AxisListType.X, op=AO.add)
                        sel2b = sb3.tile([P, nb * P], f32, tag="sel2b",
                                         name="sel2b")
                        nc.vector.tensor_tensor(
                            out=sel2b[:].rearrange("p (g j) -> p g j", g=nb),
                            in0=_bc_inner(d2s[:, u0:u0 + nb], P),
                            in1=_bc_mid(iota_f[:], nb),
                            op=AO.is_equal)
                        for j in range(nb):
                            t = chunk_tile[u0 + j]
                            k = chunk_k[u0 + j]
                            if k == 0:
                                cur_otp = ps3.tile([P, O], f32, tag="otp",
                                                   name="otp")
                            nc.tensor.matmul(
                                out=cur_otp[:],
                                lhsT=sel2b[:, j * P:(j + 1) * P],
                                rhs=m2b[:, j * O:(j + 1) * O],
                                start=(k == 0), stop=(k == k_t[t] - 1))
                            if k == k_t[t] - 1:
                                ob = sb3.tile([P, O], f32, tag="ob", name="ob")
                                nc.vector.tensor_tensor(
                                    out=ob[:], in0=cur_otp[:], in1=b2s[:],
                                    op=AO.add)
                                rows = min(P, S - t * P)
                                nc.sync.dma_start(
                                    out=outs[t * P:t * P + rows, :],
                                    in_=ob[:rows, :])

    nc.compile()
    return nc


# ---------------------------------------------------------------- entry

def _run(in_maps, plan, trace=False):
    from concourse.bass_utils import run_bass_kernel_spmd

    key = (plan["N"], plan["T1"], plan["T2"], plan["NPAIR"], DEBUG)
    nc = _CACHE.get(key)
    if nc is None:
        nc = _build(plan)
        _CACHE[key] = nc
    res = run_bass_kernel_spmd(nc, in_maps, list(range(C)), trace=trace)
    out = np.concatenate([res.results[c]["out_shard"] for c in range(C)],
                         axis=0)
    return out, res


def kernel(**inputs):
    plan, in_maps = _preprocess(**inputs)
    out, _ = _run(in_maps, plan)
    return out
